# revision 1
# baseline (speedup 1.0000x reference)
"""Trainium2 Bass kernel: LSTM encoder-decoder (IoT anomaly detector).

Reference semantics (B=256, T=512, I=128, H=256):
  encoder LSTM over x[B,T,I] -> final (h,c); pred_last = sigmoid(h @ lin_W.T + lin_b)
  decoder LSTM run T-1 steps feeding back its own prediction; outputs in
  forward time order [B,T,I].

Sharding: pure data parallelism, batch 256 -> 8 cores x 32.

Per-core layout (b=32 local batch), everything "transposed": gate/hidden
dims on SBUF partitions, batch on the free dim.  gates.T is [1024, 32] in 8
chunks of 128 partitions, stored in one PSUM tile with chunk m at cols
[32m, 32m+32), chunk order [f0 f1 i0 i1 g0 g1 o0 o1] so sigmoid(f,i) is one
ACT op over cols 0:128, tanh(g) over 128:192, sigmoid(o) over 192:256.
Weights are bf16 (fast weight load keeps LDWEIGHTS+MATMUL pairs at ~27 ns);
gate accumulation is fp32 in PSUM; c is fp32, h/pred feed back as bf16
(validated 2e-4 absmax vs the fp32 reference).  Biases enter as one matmul
per chunk: lhsT rows 0/1 = bf16 hi/lo halves of the bias, rhs = e0 (rows
0,1 = 1), so bias precision is ~fp32 at zero extra instructions on the
critical path.  The whole x shard, all weights and the preds output buffer
live in SBUF; one DMA in at start, one DMA out at the end.
"""

import numpy as np
import ml_dtypes

B, T, I, H = 256, 512, 128, 256
NCORES = 8
LB = B // NCORES  # 32 local batch

BF16 = ml_dtypes.bfloat16

_BUILT = {}


def _build(t_steps):
    import concourse.bass as bass
    import concourse.tile as tile
    from concourse import bacc, mybir

    f32 = mybir.dt.float32
    bf16 = mybir.dt.bfloat16
    AF = mybir.ActivationFunctionType
    ALU = mybir.AluOpType

    nc = bacc.Bacc(
        "TRN2", target_bir_lowering=False, debug=False, num_devices=NCORES
    )

    xT_d = nc.dram_tensor("xT", [128, t_steps * LB], bf16, kind="ExternalInput")
    wih_e_d = nc.dram_tensor("wih_e", [128, 8 * 128], bf16, kind="ExternalInput")
    whh_e_d = nc.dram_tensor("whh_e", [128, 16 * 128], bf16, kind="ExternalInput")
    bias_e_d = nc.dram_tensor("bias_e", [128, 8 * 128], bf16, kind="ExternalInput")
    wih_d_d = nc.dram_tensor("wih_d", [128, 8 * 128], bf16, kind="ExternalInput")
    whh_d_d = nc.dram_tensor("whh_d", [128, 16 * 128], bf16, kind="ExternalInput")
    bias_d_d = nc.dram_tensor("bias_d", [128, 8 * 128], bf16, kind="ExternalInput")
    wlin_d = nc.dram_tensor("wlin", [128, 2 * 128], bf16, kind="ExternalInput")
    bias_l_d = nc.dram_tensor("bias_l", [128, 128], bf16, kind="ExternalInput")
    e0_d = nc.dram_tensor("e0", [128, LB], bf16, kind="ExternalInput")
    out_d = nc.dram_tensor("out", [128, t_steps * LB], f32, kind="ExternalOutput")

    with tile.TileContext(nc) as tc:
        from contextlib import ExitStack

        with ExitStack() as ctx:
            const = ctx.enter_context(tc.tile_pool(name="const", bufs=1))
            work = ctx.enter_context(tc.tile_pool(name="work", bufs=2))
            psum = ctx.enter_context(
                tc.tile_pool(name="psum", bufs=2, space="PSUM")
            )
            psum2 = ctx.enter_context(
                tc.tile_pool(name="psum2", bufs=2, space="PSUM")
            )

            def load(dram, shape, dt):
                t = const.tile(shape, dt, tag=dram.name)
                nc.sync.dma_start(out=t[:], in_=dram[:])
                return t

            xT = load(xT_d, [128, t_steps * LB], bf16)
            wih_e = load(wih_e_d, [128, 8 * 128], bf16)
            whh_e = load(whh_e_d, [128, 16 * 128], bf16)
            bias_e = load(bias_e_d, [128, 8 * 128], bf16)
            wih_dd = load(wih_d_d, [128, 8 * 128], bf16)
            whh_dd = load(whh_d_d, [128, 16 * 128], bf16)
            bias_dd = load(bias_d_d, [128, 8 * 128], bf16)
            wlin = load(wlin_d, [128, 2 * 128], bf16)
            bias_l = load(bias_l_d, [128, 128], bf16)
            e0 = load(e0_d, [128, LB], bf16)

            preds = const.tile([128, t_steps * LB], f32, tag="preds")
            c = const.tile([128, 64], f32, tag="c")
            h = const.tile([128, 64], bf16, tag="h")
            predb = const.tile([128, LB], bf16, tag="predb")

            nc.vector.memset(c[:], 0.0)
            nc.vector.memset(h[:], 0.0)

            def cell(rhs_x, wih, whh, bias):
                ps = psum.tile([128, 256], f32, tag="gates")
                for m in range(8):
                    o = ps[:, 32 * m : 32 * m + 32]
                    nc.tensor.matmul(
                        o, bias[:, 128 * m : 128 * (m + 1)], e0[:],
                        start=True, stop=False,
                    )
                    nc.tensor.matmul(
                        o, wih[:, 128 * m : 128 * (m + 1)], rhs_x,
                        start=False, stop=False,
                    )
                    nc.tensor.matmul(
                        o, whh[:, 128 * m : 128 * (m + 1)], h[:, 0:32],
                        start=False, stop=False,
                    )
                    nc.tensor.matmul(
                        o, whh[:, 128 * (8 + m) : 128 * (9 + m)], h[:, 32:64],
                        start=False, stop=True,
                    )
                S = work.tile([128, 128], f32, tag="S")
                nc.scalar.activation(S[:], ps[:, 0:128], AF.Sigmoid)
                G = work.tile([128, 64], f32, tag="G")
                nc.scalar.activation(G[:], ps[:, 128:192], AF.Tanh)
                O = work.tile([128, 64], f32, tag="O")
                nc.scalar.activation(O[:], ps[:, 192:256], AF.Sigmoid)
                fc = work.tile([128, 64], f32, tag="fc")
                nc.vector.tensor_mul(fc[:], S[:, 0:64], c[:])
                u = work.tile([128, 64], f32, tag="u")
                nc.vector.tensor_mul(u[:], S[:, 64:128], G[:])
                nc.vector.tensor_add(c[:], fc[:], u[:])
                Tc = work.tile([128, 64], f32, tag="Tc")
                nc.scalar.activation(Tc[:], c[:], AF.Tanh)
                nc.vector.tensor_mul(h[:], O[:], Tc[:])

            def lin_block(slot):
                ps2 = psum2.tile([128, LB], f32, tag="lin")
                nc.tensor.matmul(ps2[:], bias_l[:], e0[:], start=True, stop=False)
                nc.tensor.matmul(
                    ps2[:], wlin[:, 0:128], h[:, 0:32], start=False, stop=False
                )
                nc.tensor.matmul(
                    ps2[:], wlin[:, 128:256], h[:, 32:64], start=False, stop=True
                )
                sl = preds[:, LB * slot : LB * (slot + 1)]
                nc.scalar.activation(sl, ps2[:], AF.Sigmoid)
                nc.vector.tensor_copy(predb[:], sl)

            for t in range(t_steps):
                cell(xT[:, LB * t : LB * (t + 1)], wih_e, whh_e, bias_e)
            lin_block(t_steps - 1)
            for k in range(t_steps - 1):
                cell(predb[:], wih_dd, whh_dd, bias_dd)
                lin_block(t_steps - 2 - k)

            nc.sync.dma_start(out=out_d[:], in_=preds[:])

    nc.compile()
    return nc


def _get(t_steps):
    if t_steps not in _BUILT:
        _BUILT[t_steps] = _build(t_steps)
    return _BUILT[t_steps]


def _pack_weights(enc_W_ih, enc_W_hh, enc_b_ih, enc_b_hh,
                  dec_W_ih, dec_W_hh, dec_b_ih, dec_b_hh, lin_W, lin_b):
    # chunk order [f0 f1 i0 i1 g0 g1 o0 o1]; torch gate rows are [i f g o].
    # g rows are scaled by 2 (tanh(g) = 2*sigmoid(2g) - 1).
    perm = np.r_[H : 2 * H, 0:H, 2 * H : 3 * H, 3 * H : 4 * H]

    def pack_ih(W):  # [4H, I] -> [128, 8*128] lhsT tiles
        Wp = W[perm].reshape(8, 128, I)
        return np.concatenate([Wp[m].T for m in range(8)], axis=1).astype(BF16)

    def pack_hh(W):  # [4H, H] -> [128, 16*128], tile (k,m) at col 128*(8k+m)
        Wp = W[perm]
        tiles = [
            Wp[128 * m : 128 * (m + 1), 128 * k : 128 * (k + 1)].T
            for k in range(2)
            for m in range(8)
        ]
        return np.concatenate(tiles, axis=1).astype(BF16)

    def pack_bias(b):  # [4H] -> [128, 8*128] hi/lo rows
        bp = b[perm].astype(np.float32)
        out = np.zeros((128, 8 * 128), np.float32)
        for m in range(8):
            chunk = bp[128 * m : 128 * (m + 1)]
            hi = chunk.astype(BF16).astype(np.float32)
            out[0, 128 * m : 128 * (m + 1)] = hi
            out[1, 128 * m : 128 * (m + 1)] = chunk - hi
        return out.astype(BF16)

    wlin = np.concatenate(
        [lin_W[:, 0:128].T, lin_W[:, 128:256].T], axis=1
    ).astype(BF16)
    bl = np.zeros((128, 128), np.float32)
    hi = lin_b.astype(BF16).astype(np.float32)
    bl[0, :] = hi
    bl[1, :] = lin_b - hi
    e0 = np.zeros((128, LB), np.float32)
    e0[0, :] = 1.0
    e0[1, :] = 1.0
    return {
        "wih_e": pack_ih(enc_W_ih),
        "whh_e": pack_hh(enc_W_hh),
        "bias_e": pack_bias(enc_b_ih + enc_b_hh),
        "wih_d": pack_ih(dec_W_ih),
        "whh_d": pack_hh(dec_W_hh),
        "bias_d": pack_bias(dec_b_ih + dec_b_hh),
        "wlin": wlin,
        "bias_l": bl.astype(BF16),
        "e0": e0.astype(BF16),
    }


def _run(inputs, t_steps, trace=False):
    from concourse.bass_utils import run_bass_kernel_spmd

    nc = _get(t_steps)
    x = np.asarray(inputs["x"], np.float32)
    shared = _pack_weights(
        np.asarray(inputs["enc_W_ih"], np.float32),
        np.asarray(inputs["enc_W_hh"], np.float32),
        np.asarray(inputs["enc_b_ih"], np.float32),
        np.asarray(inputs["enc_b_hh"], np.float32),
        np.asarray(inputs["dec_W_ih"], np.float32),
        np.asarray(inputs["dec_W_hh"], np.float32),
        np.asarray(inputs["dec_b_ih"], np.float32),
        np.asarray(inputs["dec_b_hh"], np.float32),
        np.asarray(inputs["lin_W"], np.float32),
        np.asarray(inputs["lin_b"], np.float32),
    )
    in_maps = []
    for j in range(NCORES):
        xs = x[LB * j : LB * (j + 1), :t_steps]  # [32, T, 128]
        xT = np.ascontiguousarray(xs.transpose(2, 1, 0)).reshape(128, t_steps * LB)
        m = dict(shared)
        m["xT"] = xT.astype(BF16)
        in_maps.append(m)

    res = run_bass_kernel_spmd(
        nc, in_maps, list(range(NCORES)), trace=trace
    )
    out = np.empty((B, t_steps, I), np.float32)
    for j in range(NCORES):
        o = res.results[j]["out"].reshape(128, t_steps, LB)
        out[LB * j : LB * (j + 1)] = o.transpose(2, 1, 0)
    return out, res


def kernel(**inputs):
    out, _ = _run(inputs, T)
    return out



# revision 3
# speedup vs baseline: 8.2387x; 8.2387x over previous
"""Trainium2 Bass kernel: LSTM encoder-decoder (IoT anomaly detector).

Reference semantics (B=256, T=512, I=128, H=256):
  encoder LSTM over x[B,T,I] -> final (h,c); pred_last = sigmoid(h @ lin_W.T + lin_b)
  decoder LSTM run T-1 steps feeding back its own prediction; outputs in
  forward time order [B,T,I].

Sharding: pure data parallelism, batch 256 -> 8 cores x 32.

v3 design (per core, local batch 32 split into 2 independent streams of 16
that software-pipeline the serial recurrence across engines):
  - transposed layout: gate/hidden dims on partitions, batch on free dim.
  - per stream+step, gates.T [8 chunks x 16 batch] accumulate in one PSUM
    bank region [128, 128], chunk order [f0 f1 i0 i1 g0 g1 o0 o1].
  - g rows of W/b pre-scaled by 2 so ONE sigmoid over all 128 cols gives
    [sig(f), sig(i), sig(2g), sig(o)]; tanh(g) = 2*sig(2g)-1 is folded into
    the DVE c-update:  P = sig(i)*sig(2g); q = 2P - sig(i) (one fused
    scalar_tensor_tensor); c = sig(f)*c + q.
  - bias enters via ONE indicator matmul per step (lhsT rows 0..7 = bf16-hi
    bias chunks, rows 8..15 = lo residual; rhs = 0/1 indicator), replacing 8
    per-chunk bias matmuls.
  - decoder linear: bias via the ACT bias AP (fp32, free); sigmoid writes
    the bf16 pred directly into the preds buffer, which is also the wih rhs
    (no CAST on the feedback path). preds DMA out as bf16; host converts.
  - PE emission order keeps h/pred-dependent matmuls last so bias/x fills of
    step t+1 run during step t's nonlinearity.
"""

import numpy as np
import ml_dtypes

B, T, I, H = 256, 512, 128, 256
NCORES = 8
LB = B // NCORES  # 32 local batch
NS = 2            # streams per core
SB = LB // NS     # 16 batch per stream

BF16 = ml_dtypes.bfloat16

# engine assignment for fc / h elementwise ops: "gpsimd" or "vector"
FC_ENGINE = "gpsimd"
H_ENGINE = "gpsimd"

_BUILT = {}


def _build(t_steps):
    import concourse.bass as bass
    import concourse.tile as tile
    from concourse import bacc, mybir

    f32 = mybir.dt.float32
    bf16 = mybir.dt.bfloat16
    AF = mybir.ActivationFunctionType
    ALU = mybir.AluOpType

    nc = bacc.Bacc(
        "TRN2", target_bir_lowering=False, debug=False, num_devices=NCORES
    )

    xT_d = nc.dram_tensor("xT", [128, t_steps * LB], bf16, kind="ExternalInput")
    wih_e_d = nc.dram_tensor("wih_e", [128, 8 * 128], bf16, kind="ExternalInput")
    whh_e_d = nc.dram_tensor("whh_e", [128, 16 * 128], bf16, kind="ExternalInput")
    bfill_e_d = nc.dram_tensor("bfill_e", [128, 128], bf16, kind="ExternalInput")
    wih_d_d = nc.dram_tensor("wih_d", [128, 8 * 128], bf16, kind="ExternalInput")
    whh_d_d = nc.dram_tensor("whh_d", [128, 16 * 128], bf16, kind="ExternalInput")
    bfill_d_d = nc.dram_tensor("bfill_d", [128, 128], bf16, kind="ExternalInput")
    wlin_d = nc.dram_tensor("wlin", [128, 2 * 128], bf16, kind="ExternalInput")
    bl_d = nc.dram_tensor("bl", [128, 1], f32, kind="ExternalInput")
    ind_d = nc.dram_tensor("ind", [128, 128], bf16, kind="ExternalInput")
    out_d = nc.dram_tensor("out", [128, t_steps * LB], bf16, kind="ExternalOutput")

    eng = {"gpsimd": nc.gpsimd, "vector": nc.vector}
    e_fc = eng[FC_ENGINE]
    e_h = eng[H_ENGINE]

    with tile.TileContext(nc) as tc:
        from contextlib import ExitStack

        with ExitStack() as ctx:
            const = ctx.enter_context(tc.tile_pool(name="const", bufs=1))
            work = ctx.enter_context(tc.tile_pool(name="work", bufs=3))
            gp = [
                ctx.enter_context(
                    tc.tile_pool(name=f"gates{s}", bufs=2, space="PSUM")
                )
                for s in range(NS)
            ]
            lp = [
                ctx.enter_context(
                    tc.tile_pool(name=f"lin{s}", bufs=1, space="PSUM")
                )
                for s in range(NS)
            ]

            def load(dram, shape, dt):
                t = const.tile(shape, dt, tag=dram.name)
                nc.sync.dma_start(out=t[:], in_=dram[:])
                return t

            xT = load(xT_d, [128, t_steps * LB], bf16)
            wih_e = load(wih_e_d, [128, 8 * 128], bf16)
            whh_e = load(whh_e_d, [128, 16 * 128], bf16)
            bfill_e = load(bfill_e_d, [128, 128], bf16)
            wih_dd = load(wih_d_d, [128, 8 * 128], bf16)
            whh_dd = load(whh_d_d, [128, 16 * 128], bf16)
            bfill_dd = load(bfill_d_d, [128, 128], bf16)
            wlin = load(wlin_d, [128, 2 * 128], bf16)
            bl = load(bl_d, [128, 1], f32)
            ind = load(ind_d, [128, 128], bf16)

            preds = const.tile([128, t_steps * LB], bf16, tag="preds")
            c = [const.tile([128, 32], f32, tag=f"c{s}", name=f"c{s}") for s in range(NS)]
            h = [const.tile([128, 32], bf16, tag=f"h{s}", name=f"h{s}") for s in range(NS)]

            for s in range(NS):
                nc.vector.memset(c[s][:], 0.0)
                nc.vector.memset(h[s][:], 0.0)

            # per-stream state: current gates psum tile
            cur_g = [None] * NS

            def pslice(t, s):
                # bf16 pred slice for time slot t, stream s
                o = LB * t + SB * s
                return preds[:, o : o + SB]

            def fill_and_x(s, t):
                """start gates(t) group: bias fill + encoder x matmuls.
                No h/pred deps -> runs during previous step's nonlin."""
                g = gp[s].tile([128, 512], f32, tag=f"g{s}")
                cur_g[s] = g
                nc.tensor.matmul(
                    g[:, 0:128], bfill_e[:], ind[:], start=True, stop=False
                )
                for m in range(8):
                    nc.tensor.matmul(
                        g[:, 16 * m : 16 * m + 16],
                        wih_e[:, 128 * m : 128 * (m + 1)],
                        xT[:, LB * t + SB * s : LB * t + SB * s + SB],
                        start=False,
                        stop=False,
                    )

            def fill_dec(s):
                """start decoder gates group: bias fill only."""
                g = gp[s].tile([128, 512], f32, tag=f"g{s}")
                cur_g[s] = g
                nc.tensor.matmul(
                    g[:, 0:128], bfill_dd[:], ind[:], start=True, stop=False
                )

            def whh_mms(s, whh, stop_at_end):
                g = cur_g[s]
                for k in range(2):
                    for m in range(8):
                        last = stop_at_end and k == 1 and m == 7
                        nc.tensor.matmul(
                            g[:, 16 * m : 16 * m + 16],
                            whh[:, 128 * (8 * k + m) : 128 * (8 * k + m + 1)],
                            h[s][:, 16 * k : 16 * k + 16],
                            start=False,
                            stop=last,
                        )

            def wih_dec_mms(s, in_slot):
                g = cur_g[s]
                rhs = pslice(in_slot, s)
                for m in range(8):
                    nc.tensor.matmul(
                        g[:, 16 * m : 16 * m + 16],
                        wih_dd[:, 128 * m : 128 * (m + 1)],
                        rhs,
                        start=False,
                        stop=(m == 7),
                    )

            def nonlin(s):
                """gates psum -> sigmoid -> c,h update."""
                g = cur_g[s]
                A = work.tile([128, 128], f32, tag=f"A{s}")
                nc.scalar.activation(A[:], g[:, 0:128], AF.Sigmoid)
                Af, Ai = A[:, 0:32], A[:, 32:64]
                Ag, Ao = A[:, 64:96], A[:, 96:128]
                fc = work.tile([128, 32], f32, tag=f"fc{s}")
                e_fc.tensor_mul(fc[:], Af, c[s][:])
                P = work.tile([128, 32], f32, tag=f"P{s}")
                nc.vector.tensor_mul(P[:], Ag, Ai)
                q = work.tile([128, 32], f32, tag=f"q{s}")
                nc.vector.scalar_tensor_tensor(
                    q[:], P[:], 2.0, Ai, ALU.mult, ALU.subtract
                )
                nc.vector.tensor_add(c[s][:], fc[:], q[:])
                Tc = work.tile([128, 32], f32, tag=f"Tc{s}")
                nc.scalar.activation(Tc[:], c[s][:], AF.Tanh)
                e_h.tensor_mul(h[s][:], Ao, Tc[:])

            def lin_block(s, out_slot):
                lz = lp[s].tile([128, 512], f32, tag=f"lz{s}")
                nc.tensor.matmul(
                    lz[:, 0:SB], wlin[:, 0:128], h[s][:, 0:16],
                    start=True, stop=False,
                )
                nc.tensor.matmul(
                    lz[:, 0:SB], wlin[:, 128:256], h[s][:, 16:32],
                    start=False, stop=True,
                )
                nc.scalar.activation(
                    pslice(out_slot, s), lz[:, 0:SB], AF.Sigmoid, bias=bl[:]
                )

            # ---- encoder ----
            for s in range(NS):
                fill_and_x(s, 0)
            for t in range(t_steps):
                for s in range(NS):
                    whh_mms(s, whh_e, stop_at_end=True)
                    nonlin(s)
                    if t + 1 < t_steps:
                        fill_and_x(s, t + 1)

            # pred at last slot from encoder final h
            for s in range(NS):
                lin_block(s, t_steps - 1)
                fill_dec(s)

            # ---- decoder: iteration k consumes pred slot T-1-k, writes T-2-k
            for k in range(t_steps - 1):
                for s in range(NS):
                    whh_mms(s, whh_dd, stop_at_end=False)
                    wih_dec_mms(s, t_steps - 1 - k)
                    nonlin(s)
                    if k + 1 < t_steps - 1:
                        fill_dec(s)
                    lin_block(s, t_steps - 2 - k)

            nc.sync.dma_start(out=out_d[:], in_=preds[:])

    nc.compile()
    return nc


def _get(t_steps):
    if t_steps not in _BUILT:
        _BUILT[t_steps] = _build(t_steps)
    return _BUILT[t_steps]


def _pack_weights(enc_W_ih, enc_W_hh, enc_b_ih, enc_b_hh,
                  dec_W_ih, dec_W_hh, dec_b_ih, dec_b_hh, lin_W, lin_b):
    # chunk order [f0 f1 i0 i1 g0 g1 o0 o1]; torch gate rows are [i f g o].
    # g rows scaled by 2: tanh(g) = 2*sigmoid(2g) - 1 folded into the merged
    # sigmoid + DVE update.
    perm = np.r_[H : 2 * H, 0:H, 2 * H : 3 * H, 3 * H : 4 * H]
    gscale = np.ones((4 * H, 1), np.float32)
    gscale[2 * H : 3 * H] = 2.0  # indexes AFTER perm: rows 2H:3H are g

    def pack_ih(W):  # [4H, I] -> [128, 8*128] lhsT tiles
        Wp = (W[perm] * gscale).reshape(8, 128, I)
        return np.concatenate([Wp[m].T for m in range(8)], axis=1).astype(BF16)

    def pack_hh(W):  # [4H, H] -> [128, 16*128], tile (k,m) at col 128*(8k+m)
        Wp = W[perm] * gscale
        tiles = [
            Wp[128 * m : 128 * (m + 1), 128 * k : 128 * (k + 1)].T
            for k in range(2)
            for m in range(8)
        ]
        return np.concatenate(tiles, axis=1).astype(BF16)

    def pack_bias_fill(b):  # [4H] -> [128, 128] lhsT: rows 0..7 hi, 8..15 lo
        bp = (b[perm] * gscale[:, 0]).astype(np.float32)
        out = np.zeros((128, 128), np.float32)
        for m in range(8):
            chunk = bp[128 * m : 128 * (m + 1)]
            hi = chunk.astype(BF16).astype(np.float32)
            out[m, :] = hi
            out[8 + m, :] = chunk - hi
        return out.astype(BF16)

    ind = np.zeros((128, 128), np.float32)
    for m in range(8):
        ind[m, 16 * m : 16 * m + 16] = 1.0
        ind[8 + m, 16 * m : 16 * m + 16] = 1.0

    wlin = np.concatenate(
        [lin_W[:, 0:128].T, lin_W[:, 128:256].T], axis=1
    ).astype(BF16)

    return {
        "wih_e": pack_ih(enc_W_ih),
        "whh_e": pack_hh(enc_W_hh),
        "bfill_e": pack_bias_fill(enc_b_ih + enc_b_hh),
        "wih_d": pack_ih(dec_W_ih),
        "whh_d": pack_hh(dec_W_hh),
        "bfill_d": pack_bias_fill(dec_b_ih + dec_b_hh),
        "wlin": wlin,
        "bl": lin_b.astype(np.float32).reshape(128, 1),
        "ind": ind.astype(BF16),
    }


def _run(inputs, t_steps, trace=False):
    from concourse.bass_utils import run_bass_kernel_spmd

    nc = _get(t_steps)
    x = np.asarray(inputs["x"], np.float32)
    shared = _pack_weights(
        np.asarray(inputs["enc_W_ih"], np.float32),
        np.asarray(inputs["enc_W_hh"], np.float32),
        np.asarray(inputs["enc_b_ih"], np.float32),
        np.asarray(inputs["enc_b_hh"], np.float32),
        np.asarray(inputs["dec_W_ih"], np.float32),
        np.asarray(inputs["dec_W_hh"], np.float32),
        np.asarray(inputs["dec_b_ih"], np.float32),
        np.asarray(inputs["dec_b_hh"], np.float32),
        np.asarray(inputs["lin_W"], np.float32),
        np.asarray(inputs["lin_b"], np.float32),
    )
    in_maps = []
    for j in range(NCORES):
        xs = x[LB * j : LB * (j + 1), :t_steps]  # [32, T, 128]
        xT = np.ascontiguousarray(xs.transpose(2, 1, 0)).reshape(128, t_steps * LB)
        m = dict(shared)
        m["xT"] = xT.astype(BF16)
        in_maps.append(m)

    res = run_bass_kernel_spmd(
        nc, in_maps, list(range(NCORES)), trace=trace
    )
    out = np.empty((B, t_steps, I), np.float32)
    for j in range(NCORES):
        o = res.results[j]["out"].astype(np.float32).reshape(128, t_steps, LB)
        out[LB * j : LB * (j + 1)] = o.transpose(2, 1, 0)
    return out, res


def kernel(**inputs):
    out, _ = _run(inputs, T)
    return out


# revision 4
# speedup vs baseline: 10.8428x; 1.3161x over previous
"""Trainium2 Bass kernel: LSTM encoder-decoder (IoT anomaly detector).

Reference semantics (B=256, T=512, I=128, H=256):
  encoder LSTM over x[B,T,I] -> final (h,c); pred_last = sigmoid(h @ lin_W.T + lin_b)
  decoder LSTM run T-1 steps feeding back its own prediction; outputs in
  forward time order [B,T,I].

Sharding: pure data parallelism, batch 256 -> 8 cores x 32.

v4: the recurrence is latency-bound (one dependency cycle per time step);
minimize the cycle:
  - transposed layout: gate/hidden dims on partitions, batch (32) on free.
  - gates split across TWO psum banks: bank A = [f0 f1 i0 i1] (cols 4x32),
    bank B = [g0 g1 o0 o1].  sigmoid(f,i) waits only on bank A's 8 whh
    matmuls; bank B's 8 whh matmuls run while sigmoid(f,i) executes.
  - nonlin chain (all elementwise on DVE, in-order): fc = sig_f*c (runs
    under ACT tanh_g), u = sig_i*G, c = fc+u, ACT tanh_c, h = sig_o*Tc
    (bf16 out, feeds next step's matmuls directly).
  - bias enters via ONE indicator matmul per bank (lhsT rows 0..3 hi /
    8..11 lo bf16 split), emitted with the x matmuls BEFORE the
    h-dependent whh matmuls so they execute during the previous step's
    nonlinearity.
  - decoder linear: z matmuls emitted right after h; sigmoid (bias via the
    fp32 ACT bias AP) writes the bf16 pred directly into the preds buffer,
    which is also the wih rhs.  preds DMA out as bf16; host converts.
"""

import numpy as np
import ml_dtypes

B, T, I, H = 256, 512, 128, 256
NCORES = 8
LB = B // NCORES  # 32 local batch

BF16 = ml_dtypes.bfloat16

_BUILT = {}


def _build(t_steps):
    import concourse.bass as bass
    import concourse.tile as tile
    from concourse import bacc, mybir

    f32 = mybir.dt.float32
    bf16 = mybir.dt.bfloat16
    AF = mybir.ActivationFunctionType

    nc = bacc.Bacc(
        "TRN2", target_bir_lowering=False, debug=False, num_devices=NCORES
    )

    xT_d = nc.dram_tensor("xT", [128, t_steps * LB], bf16, kind="ExternalInput")
    wih_e_d = nc.dram_tensor("wih_e", [128, 8 * 128], bf16, kind="ExternalInput")
    whh_e_d = nc.dram_tensor("whh_e", [128, 16 * 128], bf16, kind="ExternalInput")
    bfa_e_d = nc.dram_tensor("bfa_e", [128, 128], bf16, kind="ExternalInput")
    bfb_e_d = nc.dram_tensor("bfb_e", [128, 128], bf16, kind="ExternalInput")
    wih_d_d = nc.dram_tensor("wih_d", [128, 8 * 128], bf16, kind="ExternalInput")
    whh_d_d = nc.dram_tensor("whh_d", [128, 16 * 128], bf16, kind="ExternalInput")
    bfa_d_d = nc.dram_tensor("bfa_d", [128, 128], bf16, kind="ExternalInput")
    bfb_d_d = nc.dram_tensor("bfb_d", [128, 128], bf16, kind="ExternalInput")
    wlin_d = nc.dram_tensor("wlin", [128, 2 * 128], bf16, kind="ExternalInput")
    bl_d = nc.dram_tensor("bl", [128, 1], f32, kind="ExternalInput")
    ind_d = nc.dram_tensor("ind", [128, 128], bf16, kind="ExternalInput")
    out_d = nc.dram_tensor("out", [128, t_steps * LB], bf16, kind="ExternalOutput")

    with tile.TileContext(nc) as tc:
        from contextlib import ExitStack

        with ExitStack() as ctx:
            const = ctx.enter_context(tc.tile_pool(name="const", bufs=1))
            work = ctx.enter_context(tc.tile_pool(name="work", bufs=3))
            pa = ctx.enter_context(tc.tile_pool(name="pa", bufs=2, space="PSUM"))
            pb = ctx.enter_context(tc.tile_pool(name="pb", bufs=2, space="PSUM"))
            pl = ctx.enter_context(tc.tile_pool(name="pl", bufs=2, space="PSUM"))

            def load(dram, shape, dt):
                t = const.tile(shape, dt, tag=dram.name)
                nc.sync.dma_start(out=t[:], in_=dram[:])
                return t

            xT = load(xT_d, [128, t_steps * LB], bf16)
            wih_e = load(wih_e_d, [128, 8 * 128], bf16)
            whh_e = load(whh_e_d, [128, 16 * 128], bf16)
            bfa_e = load(bfa_e_d, [128, 128], bf16)
            bfb_e = load(bfb_e_d, [128, 128], bf16)
            wih_dd = load(wih_d_d, [128, 8 * 128], bf16)
            whh_dd = load(whh_d_d, [128, 16 * 128], bf16)
            bfa_dd = load(bfa_d_d, [128, 128], bf16)
            bfb_dd = load(bfb_d_d, [128, 128], bf16)
            wlin = load(wlin_d, [128, 2 * 128], bf16)
            bl = load(bl_d, [128, 1], f32)
            ind = load(ind_d, [128, 128], bf16)

            preds = const.tile([128, t_steps * LB], bf16, tag="preds")
            c = const.tile([128, 64], f32, tag="c")
            h = const.tile([128, 64], bf16, tag="h")

            nc.vector.memset(c[:], 0.0)
            nc.vector.memset(h[:], 0.0)

            cur = {"A": None, "B": None}

            def pslice(t):
                return preds[:, LB * t : LB * (t + 1)]

            def fills(bfa, bfb, x_t=None, wih=None):
                """open gates(t) groups on both banks: bias fill (+ encoder
                x matmuls). No h/pred deps -> run during previous nonlin."""
                ga = pa.tile([128, 512], f32, tag="ga")
                gb = pb.tile([128, 512], f32, tag="gb")
                cur["A"], cur["B"] = ga, gb
                nc.tensor.matmul(ga[:, 0:128], bfa[:], ind[:], start=True, stop=False)
                nc.tensor.matmul(gb[:, 0:128], bfb[:], ind[:], start=True, stop=False)
                if x_t is not None:
                    rhs = xT[:, LB * x_t : LB * (x_t + 1)]
                    for m in range(8):
                        g = cur["A" if m < 4 else "B"]
                        nc.tensor.matmul(
                            g[:, 32 * (m % 4) : 32 * (m % 4) + 32],
                            wih[:, 128 * m : 128 * (m + 1)],
                            rhs,
                            start=False,
                            stop=False,
                        )

            def whh_mms(whh, bank, stop):
                g = cur[bank]
                ms = range(0, 4) if bank == "A" else range(4, 8)
                for k in range(2):
                    for j, m in enumerate(ms):
                        last = stop and k == 1 and j == 3
                        nc.tensor.matmul(
                            g[:, 32 * (m % 4) : 32 * (m % 4) + 32],
                            whh[:, 128 * (8 * k + m) : 128 * (8 * k + m + 1)],
                            h[:, 32 * k : 32 * k + 32],
                            start=False,
                            stop=last,
                        )

            def wih_mms(wih, in_slot, bank):
                g = cur[bank]
                rhs = pslice(in_slot)
                ms = range(0, 4) if bank == "A" else range(4, 8)
                for j, m in enumerate(ms):
                    nc.tensor.matmul(
                        g[:, 32 * (m % 4) : 32 * (m % 4) + 32],
                        wih[:, 128 * m : 128 * (m + 1)],
                        rhs,
                        start=False,
                        stop=(j == 3),
                    )

            def nonlin():
                ga, gb = cur["A"], cur["B"]
                S = work.tile([128, 128], f32, tag="S")
                nc.scalar.activation(S[:], ga[:, 0:128], AF.Sigmoid)
                G = work.tile([128, 64], f32, tag="G")
                nc.scalar.activation(G[:], gb[:, 0:64], AF.Tanh)
                O = work.tile([128, 64], f32, tag="O")
                nc.scalar.activation(O[:], gb[:, 64:128], AF.Sigmoid)
                fc = work.tile([128, 64], f32, tag="fc")
                nc.vector.tensor_mul(fc[:], S[:, 0:64], c[:])
                u = work.tile([128, 64], f32, tag="u")
                nc.vector.tensor_mul(u[:], S[:, 64:128], G[:])
                nc.vector.tensor_add(c[:], fc[:], u[:])
                Tc = work.tile([128, 64], f32, tag="Tc")
                nc.scalar.activation(Tc[:], c[:], AF.Tanh)
                nc.vector.tensor_mul(h[:], O[:], Tc[:])

            def lin_block(out_slot):
                lz = pl.tile([128, 512], f32, tag="lz")
                nc.tensor.matmul(
                    lz[:, 0:LB], wlin[:, 0:128], h[:, 0:32],
                    start=True, stop=False,
                )
                nc.tensor.matmul(
                    lz[:, 0:LB], wlin[:, 128:256], h[:, 32:64],
                    start=False, stop=True,
                )
                nc.scalar.activation(
                    pslice(out_slot), lz[:, 0:LB], AF.Sigmoid, bias=bl[:]
                )

            # ---- encoder ----
            fills(bfa_e, bfb_e, x_t=0, wih=wih_e)
            for t in range(t_steps):
                whh_mms(whh_e, "A", stop=True)
                whh_mms(whh_e, "B", stop=True)
                nonlin()
                if t + 1 < t_steps:
                    fills(bfa_e, bfb_e, x_t=t + 1, wih=wih_e)

            # pred at last slot from encoder final h; open first decoder banks
            fills(bfa_dd, bfb_dd)
            lin_block(t_steps - 1)

            # ---- decoder: iteration k consumes pred slot T-1-k, writes T-2-k
            for k in range(t_steps - 1):
                whh_mms(whh_dd, "A", stop=False)
                whh_mms(whh_dd, "B", stop=False)
                wih_mms(wih_dd, t_steps - 1 - k, "A")
                wih_mms(wih_dd, t_steps - 1 - k, "B")
                nonlin()
                if k + 1 < t_steps - 1:
                    fills(bfa_dd, bfb_dd)
                lin_block(t_steps - 2 - k)

            nc.sync.dma_start(out=out_d[:], in_=preds[:])

    nc.compile()
    return nc


def _get(t_steps):
    if t_steps not in _BUILT:
        _BUILT[t_steps] = _build(t_steps)
    return _BUILT[t_steps]


def _pack_weights(enc_W_ih, enc_W_hh, enc_b_ih, enc_b_hh,
                  dec_W_ih, dec_W_hh, dec_b_ih, dec_b_hh, lin_W, lin_b):
    # chunk order [f0 f1 i0 i1 g0 g1 o0 o1]; torch gate rows are [i f g o].
    perm = np.r_[H : 2 * H, 0:H, 2 * H : 3 * H, 3 * H : 4 * H]

    def pack_ih(W):  # [4H, I] -> [128, 8*128] lhsT tiles
        Wp = W[perm].reshape(8, 128, I)
        return np.concatenate([Wp[m].T for m in range(8)], axis=1).astype(BF16)

    def pack_hh(W):  # [4H, H] -> [128, 16*128], tile (k,m) at col 128*(8k+m)
        Wp = W[perm]
        tiles = [
            Wp[128 * m : 128 * (m + 1), 128 * k : 128 * (k + 1)].T
            for k in range(2)
            for m in range(8)
        ]
        return np.concatenate(tiles, axis=1).astype(BF16)

    def pack_bias_fill(b, bank):  # [4H] -> [128,128]: rows 0..3 hi, 8..11 lo
        bp = b[perm].astype(np.float32)
        out = np.zeros((128, 128), np.float32)
        base = 0 if bank == "A" else 4
        for j in range(4):
            chunk = bp[128 * (base + j) : 128 * (base + j + 1)]
            hi = chunk.astype(BF16).astype(np.float32)
            out[j, :] = hi
            out[8 + j, :] = chunk - hi
        return out.astype(BF16)

    ind = np.zeros((128, 128), np.float32)
    for j in range(4):
        ind[j, 32 * j : 32 * j + 32] = 1.0
        ind[8 + j, 32 * j : 32 * j + 32] = 1.0

    wlin = np.concatenate(
        [lin_W[:, 0:128].T, lin_W[:, 128:256].T], axis=1
    ).astype(BF16)

    b_e = enc_b_ih + enc_b_hh
    b_d = dec_b_ih + dec_b_hh
    return {
        "wih_e": pack_ih(enc_W_ih),
        "whh_e": pack_hh(enc_W_hh),
        "bfa_e": pack_bias_fill(b_e, "A"),
        "bfb_e": pack_bias_fill(b_e, "B"),
        "wih_d": pack_ih(dec_W_ih),
        "whh_d": pack_hh(dec_W_hh),
        "bfa_d": pack_bias_fill(b_d, "A"),
        "bfb_d": pack_bias_fill(b_d, "B"),
        "wlin": wlin,
        "bl": lin_b.astype(np.float32).reshape(128, 1),
        "ind": ind.astype(BF16),
    }


def _run(inputs, t_steps, trace=False):
    from concourse.bass_utils import run_bass_kernel_spmd

    nc = _get(t_steps)
    x = np.asarray(inputs["x"], np.float32)
    shared = _pack_weights(
        np.asarray(inputs["enc_W_ih"], np.float32),
        np.asarray(inputs["enc_W_hh"], np.float32),
        np.asarray(inputs["enc_b_ih"], np.float32),
        np.asarray(inputs["enc_b_hh"], np.float32),
        np.asarray(inputs["dec_W_ih"], np.float32),
        np.asarray(inputs["dec_W_hh"], np.float32),
        np.asarray(inputs["dec_b_ih"], np.float32),
        np.asarray(inputs["dec_b_hh"], np.float32),
        np.asarray(inputs["lin_W"], np.float32),
        np.asarray(inputs["lin_b"], np.float32),
    )
    in_maps = []
    for j in range(NCORES):
        xs = x[LB * j : LB * (j + 1), :t_steps]  # [32, T, 128]
        xT = np.ascontiguousarray(xs.transpose(2, 1, 0)).reshape(128, t_steps * LB)
        m = dict(shared)
        m["xT"] = xT.astype(BF16)
        in_maps.append(m)

    res = run_bass_kernel_spmd(
        nc, in_maps, list(range(NCORES)), trace=trace
    )
    out = np.empty((B, t_steps, I), np.float32)
    for j in range(NCORES):
        o = res.results[j]["out"].astype(np.float32).reshape(128, t_steps, LB)
        out[LB * j : LB * (j + 1)] = o.transpose(2, 1, 0)
    return out, res


def kernel(**inputs):
    out, _ = _run(inputs, T)
    return out
